# revision 20
# baseline (speedup 1.0000x reference)
"""ClusterGCN layer on 8 TRN2 NeuronCores.

Math: for each cluster c (only intra-cluster edges matter),
    Y_c = B_c @ (X_c @ W) + b
    B_c[d, s] = dis[d] * At_c[s, d] * dis[s]
    At_c[s, d] = #edges(s->d in c) + [s == d]     (self-loop: dis^2 = 1/deg)
with dis = rsqrt(deg), deg = intra in-degree + 1. Clusters with no intra
edge pass X through unchanged (patched on host).

Device per core (pipelined over clusters), all matmuls fp16 on the PE
(fp16 matches bf16 PE throughput with 4x the mantissa; values are O(1)):
  step1: xws = (X @ W) * dis[s]   -- nodes on partitions; the dis scale is
         folded into the PSUM->SBUF cast on the Scalar engine.
  step2: Z_c^T = xws^T-chunks (stationary) x At_c (moving, N=cap), i.e.
         Z^T[f, d] = sum_s xws[s,f] * At[s,d];  Y_c = dis[d] * Z[d] (host).
At ships as fp8e4m3 edge counts (integer counts <= 16 are exact in
e4m3, and the PE accepts a mixed fp16-stationary x fp8-moving matmul).
Output is per-cluster transposed [f, cap]; host de-transposes in the
gather, applies dis[d] and the bias.
"""

import os

import numpy as np

N_CORES = 8
N_CLUSTERS = 100
P = 128

# compute dtype for X/W/xW tiles: fp16 matches bf16 PE throughput with 4x
# the mantissa (all values here are O(1), so fp16 range is safe)
_X_DT = os.environ.get("KX_DTYPE", "fp16")
_Y_DT = os.environ.get("KYT_DTYPE", "fp16")

_prog_cache: dict = {}


def _build_program(cpc: int, cap: int, in_c: int, f_out: int, fp8_path: bool,
                   dmax: int = 0):
    """Build + compile the per-core Bass program.

    cpc: clusters per core; cap: padded cluster size (multiple of 128).
    fp8_path: adjacency as fp8e4m3 counts + on-device scaling (default);
    the bf16 fallback ships pre-scaled B^T blocks (counts > 16 only).
    """
    import concourse.mybir as mybir
    import concourse.tile as tile
    from concourse import bacc

    dmax = dmax or cap
    key = (cpc, cap, in_c, f_out, fp8_path, _X_DT, _Y_DT, dmax)
    if key in _prog_cache:
        return _prog_cache[key]

    kc = in_c // P           # contraction chunks for X @ W
    sch = cap // P           # s chunks per cluster
    fc = f_out // P          # f chunks (step-2 output partitions)
    f32 = mybir.dt.float32
    f32r = mybir.dt.float32r
    dt_of = {"bf16": mybir.dt.bfloat16, "fp16": mybir.dt.float16,
             "f32": f32, "f32r": f32r}
    fp8 = mybir.dt.float8e4
    x_dt = dt_of[_X_DT]
    a_dt = fp8 if fp8_path else x_dt

    XG = 4                   # clusters per XT load group
    AG = 2                   # clusters per At load group

    nc = bacc.Bacc("TRN2", target_bir_lowering=False, debug=False,
                   num_devices=N_CORES)

    XT = nc.dram_tensor("XT", [in_c, cpc * cap], x_dt, kind="ExternalInput")
    Wt = nc.dram_tensor("Wt", [in_c, f_out], x_dt, kind="ExternalInput")
    AT = nc.dram_tensor("AT", [cpc, P, sch, cap], a_dt, kind="ExternalInput")
    DIS = nc.dram_tensor("DIS", [P, cpc, sch], f32, kind="ExternalInput")
    y_dt = dt_of[_Y_DT]
    YT = nc.dram_tensor("YT", [cpc, f_out, dmax], y_dt, kind="ExternalOutput")

    XTr = XT.rearrange("(k p) n -> p k n", p=P)

    with tile.TileContext(nc) as tc:
        with (
            tc.tile_pool(name="w", bufs=1) as w_pool,
            tc.tile_pool(name="xt", bufs=3) as xt_pool,
            tc.tile_pool(name="at", bufs=4) as at_pool,
            tc.tile_pool(name="xw", bufs=4 * sch) as xw_pool,
            tc.tile_pool(name="out", bufs=5) as out_pool,
            tc.tile_pool(name="ps1", bufs=4, space="PSUM") as ps1_pool,
            tc.tile_pool(name="ps2", bufs=4, space="PSUM") as ps2_pool,
        ):
            # small loads go on the Scalar queue so the Sync queue's first
            # work is cluster 0's data
            wt = w_pool.tile([P, kc, f_out], x_dt)
            nc.scalar.dma_start(wt[:], Wt.rearrange("(k p) f -> p k f", p=P))
            dcol = w_pool.tile([P, cpc, sch], f32)
            nc.scalar.dma_start(dcol[:], DIS[:])


            # group sizes: a small first load so compute starts early
            def groups(g):
                sizes, c0 = [], 0
                first = True
                while c0 < cpc:
                    g_ = 1 if first else min(g, cpc - c0)
                    sizes.append((c0, min(g_, cpc - c0)))
                    c0 += sizes[-1][1]
                    first = False
                return sizes

            xg_of = {}
            for c0, g in groups(XG):
                for c in range(c0, c0 + g):
                    xg_of[c] = (c0, g)
            ag_of = {}
            for c0, g in groups(AG):
                for c in range(c0, c0 + g):
                    ag_of[c] = (c0, g)

            xt = at = None
            for c in range(cpc):
                c0, g = xg_of[c]
                if c == c0:
                    xt = xt_pool.tile([P, kc, XG * cap], x_dt)
                    nc.sync.dma_start(
                        xt[:, :, :g * cap],
                        XTr[:, :, c0 * cap:(c0 + g) * cap],
                    )
                a0, ag = ag_of[c]
                if c == a0:
                    at = at_pool.tile([P, AG, sch, cap], a_dt)
                    nc.sync.dma_start(
                        at[:, :ag],
                        AT[a0:a0 + ag].rearrange("c p so d -> p c so d"),
                    )
                xoff = (c - c0) * cap
                ci = c - a0

                ab = at[:, ci]

                # step 1: xws = (X @ W) * dis, nodes on partitions
                xw_tiles = []
                for t in range(sch):
                    ps = ps1_pool.tile([P, f_out], f32)
                    for k in range(kc):
                        nc.tensor.matmul(
                            ps[:],
                            lhsT=xt[:, k, xoff + t * P:xoff + (t + 1) * P],
                            rhs=wt[:, k, :],
                            start=(k == 0),
                            stop=(k == kc - 1),
                        )
                    xw = xw_pool.tile([P, f_out], x_dt)
                    if fp8_path:
                        nc.scalar.activation(
                            xw[:], ps[:], mybir.ActivationFunctionType.Copy,
                            scale=dcol[:, c, t:t + 1],
                        )
                    else:
                        nc.scalar.copy(xw[:], ps[:])
                    xw_tiles.append(xw)

                # step 2: Z_c^T = sum_s xws-chunk^T x At rows, f on
                # partitions; d chunked to the 512-fp32 PSUM bank limit
                ot = out_pool.tile([P, fc, dmax], y_dt)
                for f in range(fc):
                    for d0 in range(0, dmax, 512):
                        dn = min(512, dmax - d0)
                        ps = ps2_pool.tile([P, 512], f32)
                        for s in range(sch):
                            nc.tensor.matmul(
                                ps[:, :dn],
                                lhsT=xw_tiles[s][:, f * P:(f + 1) * P],
                                rhs=ab[:, s, d0:d0 + dn],
                                start=(s == 0),
                                stop=(s == sch - 1),
                            )
                        nc.vector.tensor_copy(
                            ot[:, f, d0:d0 + dn], ps[:, :dn])
                nc.sync.dma_start(
                    YT[c].rearrange("(f p) d -> p f d", p=P), ot[:]
                )

    nc.compile()
    _prog_cache[key] = nc
    return nc


def _host_prep(X, W, b, assign, full_ei):
    """Shard + preprocess. Returns (in_maps, fp8_path, gather info)."""
    n, in_c = X.shape
    f_out = W.shape[1]
    src = full_ei[0].astype(np.int64)
    dst = full_ei[1].astype(np.int64)
    a_s = assign[src]
    intra = a_s == assign[dst]
    es, ed = src[intra], dst[intra]

    deg = np.ones(n, np.float32)
    np.add.at(deg, ed, np.float32(1))
    dis = (1.0 / np.sqrt(deg)).astype(np.float32)

    has_edge = np.zeros(N_CLUSTERS, bool)
    has_edge[np.unique(a_s[intra])] = True

    sizes = np.bincount(assign, minlength=N_CLUSTERS)
    cpc = -(-N_CLUSTERS // N_CORES)            # clusters per core
    cap = max(512, int(-(-sizes.max() // P)) * P)  # padded cluster size

    starts = np.zeros(N_CLUSTERS + 1, np.int64)
    starts[1:] = np.cumsum(sizes)
    order = np.argsort(assign, kind="stable")
    pos = np.empty(n, np.int64)
    pos[order] = np.arange(n) - starts[assign[order]]

    ctot = cpc * N_CORES
    # At blocks: At[c][s, d] = #edges(s->d) + [s==d]
    At = np.zeros((ctot, cap, cap), np.uint16)
    np.add.at(At, (assign[es], pos[es], pos[ed]), 1)
    At[assign, pos, pos] += 1
    fp8_path = int(At.max()) <= 16    # integers <= 16 are exact in e4m3

    import ml_dtypes
    x_np = {"bf16": ml_dtypes.bfloat16, "fp16": np.float16,
            "f32": np.float32, "f32r": np.float32}[_X_DT]

    Xp = np.zeros((ctot, cap, in_c), np.float32)
    Xp[assign, pos] = X
    XT_all = np.ascontiguousarray(Xp.reshape(ctot * cap, in_c).T)

    DISp = np.zeros((ctot, cap), np.float32)
    DISp[assign, pos] = dis

    if fp8_path:
        import concourse.mybir as mybir
        At_send = At.astype(mybir.dt.np(mybir.dt.float8e4))
    else:
        # rare fallback: pre-scaled B^T blocks in the compute dtype
        At_send = (At.astype(np.float32)
                   * DISp[:, :, None] * DISp[:, None, :]).astype(x_np)
    # [c, s, d] -> [c, p, so, d] so each partition row is one 2KB run
    sch = cap // P
    At_send = np.ascontiguousarray(
        At_send.reshape(-1, sch, P, cap).transpose(0, 2, 1, 3))
    DIS_send = np.ascontiguousarray(
        DISp.reshape(-1, sch, P).transpose(2, 0, 1))  # [128, ctot, sch]

    nodes = cpc * cap
    in_maps = []
    for i in range(N_CORES):
        in_maps.append({
            "XT": np.ascontiguousarray(
                XT_all[:, i * nodes:(i + 1) * nodes]).astype(x_np),
            "Wt": W.astype(np.float32).astype(x_np),
            "AT": At_send[i * cpc:(i + 1) * cpc],
            "DIS": np.ascontiguousarray(
                DIS_send[:, i * cpc:(i + 1) * cpc]),
        })
    dmax = int(sizes.max())
    return in_maps, fp8_path, (cpc, cap, dmax, has_edge, pos, dis)


def _run(inputs, trace=False, tmpdir=None):
    from concourse.bass_utils import run_bass_kernel_spmd

    X = np.asarray(inputs["X"], np.float32)
    W = np.asarray(inputs["W"], np.float32)
    b = np.asarray(inputs["b"], np.float32)
    assign = np.asarray(inputs["assign"])
    full_ei = np.asarray(inputs["full_ei"])

    n, in_c = X.shape
    f_out = W.shape[1]
    in_maps, fp8_path, (cpc, cap, dmax, has_edge, pos, dis) = _host_prep(
        X, W, b, assign, full_ei)
    nc = _build_program(cpc, cap, in_c, f_out, fp8_path, dmax)

    res = run_bass_kernel_spmd(
        nc, in_maps, core_ids=list(range(N_CORES)),
        trace=trace, tmpdir=tmpdir,
    )
    # YT: [core][cpc, f_out, cap]; row n lives at [core, lc, :, pos]
    YTdev = np.stack([res.results[i]["YT"] for i in range(N_CORES)])
    if YTdev.dtype != np.float32:
        YTdev = YTdev.astype(np.float32)

    c = assign.astype(np.int64)
    core = c // cpc
    lc = c % cpc
    Y = YTdev[core, lc, :, pos]
    if fp8_path:
        Y *= dis[:, None]
    Y += b[None, :].astype(np.float32)
    miss = ~has_edge[c]
    if miss.any():
        Y[miss] = X[miss]
    return Y, res


def kernel(**inputs) -> np.ndarray:
    Y, _ = _run(inputs)
    return Y


# revision 21
# speedup vs baseline: 1.0376x; 1.0376x over previous
"""ClusterGCN layer on 8 TRN2 NeuronCores.

Math: for each cluster c (only intra-cluster edges matter),
    Y_c = B_c @ (X_c @ W) + b
    B_c[d, s] = dis[d] * At_c[s, d] * dis[s]
    At_c[s, d] = #edges(s->d in c) + [s == d]     (self-loop: dis^2 = 1/deg)
with dis = rsqrt(deg), deg = intra in-degree + 1. Clusters with no intra
edge pass X through unchanged (patched on host).

Device per core (pipelined over clusters), all matmuls fp16 on the PE
(fp16 matches bf16 PE throughput with 4x the mantissa; values are O(1)):
  step1: xws = (X @ W) * dis[s]   -- nodes on partitions; the dis scale is
         folded into the PSUM->SBUF cast on the Scalar engine.
  step2: Z_c^T = xws^T-chunks (stationary) x At_c (moving, N=cap), i.e.
         Z^T[f, d] = sum_s xws[s,f] * At[s,d];  Y_c = dis[d] * Z[d] (host).
At ships as fp8e4m3 edge counts (integer counts <= 16 are exact in
e4m3, and the PE accepts a mixed fp16-stationary x fp8-moving matmul).
Output is per-cluster transposed [f, cap]; host de-transposes in the
gather, applies dis[d] and the bias.
"""

import os

import numpy as np

N_CORES = 8
N_CLUSTERS = 100
P = 128

# compute dtype for X/W/xW tiles: fp16 matches bf16 PE throughput with 4x
# the mantissa (all values here are O(1), so fp16 range is safe)
_X_DT = os.environ.get("KX_DTYPE", "fp16")
_Y_DT = os.environ.get("KYT_DTYPE", "fp16")

_prog_cache: dict = {}


def _build_program(cpc: int, cap: int, in_c: int, f_out: int, fp8_path: bool):
    """Build + compile the per-core Bass program.

    cpc: clusters per core; cap: padded cluster size (multiple of 128).
    fp8_path: adjacency as fp8e4m3 counts + on-device scaling (default);
    the bf16 fallback ships pre-scaled B^T blocks (counts > 16 only).
    """
    import concourse.mybir as mybir
    import concourse.tile as tile
    from concourse import bacc

    key = (cpc, cap, in_c, f_out, fp8_path, _X_DT, _Y_DT)
    if key in _prog_cache:
        return _prog_cache[key]

    kc = in_c // P           # contraction chunks for X @ W
    sch = cap // P           # s chunks per cluster
    fc = f_out // P          # f chunks (step-2 output partitions)
    f32 = mybir.dt.float32
    f32r = mybir.dt.float32r
    dt_of = {"bf16": mybir.dt.bfloat16, "fp16": mybir.dt.float16,
             "f32": f32, "f32r": f32r}
    fp8 = mybir.dt.float8e4
    x_dt = dt_of[_X_DT]
    a_dt = fp8 if fp8_path else x_dt

    XG = 4                   # clusters per XT load group
    AG = 2                   # clusters per At load group

    nc = bacc.Bacc("TRN2", target_bir_lowering=False, debug=False,
                   num_devices=N_CORES)

    XT = nc.dram_tensor("XT", [in_c, cpc * cap], x_dt, kind="ExternalInput")
    Wt = nc.dram_tensor("Wt", [in_c, f_out], x_dt, kind="ExternalInput")
    AT = nc.dram_tensor("AT", [cpc, P, sch, cap], a_dt, kind="ExternalInput")
    DIS = nc.dram_tensor("DIS", [P, cpc, sch], f32, kind="ExternalInput")
    y_dt = dt_of[_Y_DT]
    YT = nc.dram_tensor("YT", [cpc, f_out, cap], y_dt, kind="ExternalOutput")

    XTr = XT.rearrange("(k p) n -> p k n", p=P)

    with tile.TileContext(nc) as tc:
        with (
            tc.tile_pool(name="w", bufs=1) as w_pool,
            tc.tile_pool(name="xt", bufs=3) as xt_pool,
            tc.tile_pool(name="at", bufs=4) as at_pool,
            tc.tile_pool(name="xw", bufs=4 * sch) as xw_pool,
            tc.tile_pool(name="out", bufs=5) as out_pool,
            tc.tile_pool(name="ps1", bufs=4, space="PSUM") as ps1_pool,
            tc.tile_pool(name="ps2", bufs=4, space="PSUM") as ps2_pool,
        ):
            # small loads go on the Scalar queue so the Sync queue's first
            # work is cluster 0's data
            wt = w_pool.tile([P, kc, f_out], x_dt)
            nc.scalar.dma_start(wt[:], Wt.rearrange("(k p) f -> p k f", p=P))
            dcol = w_pool.tile([P, cpc, sch], f32)
            nc.scalar.dma_start(dcol[:], DIS[:])


            # group sizes: a small first load so compute starts early
            def groups(g):
                sizes, c0 = [], 0
                first = True
                while c0 < cpc:
                    g_ = 1 if first else min(g, cpc - c0)
                    sizes.append((c0, min(g_, cpc - c0)))
                    c0 += sizes[-1][1]
                    first = False
                return sizes

            xg_of = {}
            for c0, g in groups(XG):
                for c in range(c0, c0 + g):
                    xg_of[c] = (c0, g)
            ag_of = {}
            for c0, g in groups(AG):
                for c in range(c0, c0 + g):
                    ag_of[c] = (c0, g)

            xt = at = None
            for c in range(cpc):
                c0, g = xg_of[c]
                if c == c0:
                    xt = xt_pool.tile([P, kc, XG * cap], x_dt)
                    nc.sync.dma_start(
                        xt[:, :, :g * cap],
                        XTr[:, :, c0 * cap:(c0 + g) * cap],
                    )
                a0, ag = ag_of[c]
                if c == a0:
                    at = at_pool.tile([P, AG, sch, cap], a_dt)
                    nc.sync.dma_start(
                        at[:, :ag],
                        AT[a0:a0 + ag].rearrange("c p so d -> p c so d"),
                    )
                xoff = (c - c0) * cap
                ci = c - a0

                ab = at[:, ci]

                # step 1: xws = (X @ W) * dis, nodes on partitions
                xw_tiles = []
                for t in range(sch):
                    ps = ps1_pool.tile([P, f_out], f32)
                    for k in range(kc):
                        nc.tensor.matmul(
                            ps[:],
                            lhsT=xt[:, k, xoff + t * P:xoff + (t + 1) * P],
                            rhs=wt[:, k, :],
                            start=(k == 0),
                            stop=(k == kc - 1),
                        )
                    xw = xw_pool.tile([P, f_out], x_dt)
                    if fp8_path:
                        nc.scalar.activation(
                            xw[:], ps[:], mybir.ActivationFunctionType.Copy,
                            scale=dcol[:, c, t:t + 1],
                        )
                    else:
                        nc.scalar.copy(xw[:], ps[:])
                    xw_tiles.append(xw)

                # step 2: Z_c^T = sum_s xws-chunk^T x At rows, f on
                # partitions; d chunked to the 512-fp32 PSUM bank limit
                ot = out_pool.tile([P, fc, cap], y_dt)
                for f in range(fc):
                    for d0 in range(0, cap, 512):
                        dn = min(512, cap - d0)
                        ps = ps2_pool.tile([P, 512], f32)
                        for s in range(sch):
                            nc.tensor.matmul(
                                ps[:, :dn],
                                lhsT=xw_tiles[s][:, f * P:(f + 1) * P],
                                rhs=ab[:, s, d0:d0 + dn],
                                start=(s == 0),
                                stop=(s == sch - 1),
                            )
                        nc.vector.tensor_copy(
                            ot[:, f, d0:d0 + dn], ps[:, :dn])
                nc.sync.dma_start(
                    YT[c].rearrange("(f p) d -> p f d", p=P), ot[:]
                )

    nc.compile()
    _prog_cache[key] = nc
    return nc


def _host_prep(X, W, b, assign, full_ei):
    """Shard + preprocess. Returns (in_maps, fp8_path, gather info)."""
    n, in_c = X.shape
    f_out = W.shape[1]
    src = full_ei[0].astype(np.int64)
    dst = full_ei[1].astype(np.int64)
    a_s = assign[src]
    intra = a_s == assign[dst]
    es, ed = src[intra], dst[intra]

    deg = np.ones(n, np.float32)
    np.add.at(deg, ed, np.float32(1))
    dis = (1.0 / np.sqrt(deg)).astype(np.float32)

    has_edge = np.zeros(N_CLUSTERS, bool)
    has_edge[np.unique(a_s[intra])] = True

    sizes = np.bincount(assign, minlength=N_CLUSTERS)
    cpc = -(-N_CLUSTERS // N_CORES)            # clusters per core
    cap = max(512, int(-(-sizes.max() // P)) * P)  # padded cluster size

    starts = np.zeros(N_CLUSTERS + 1, np.int64)
    starts[1:] = np.cumsum(sizes)
    order = np.argsort(assign, kind="stable")
    pos = np.empty(n, np.int64)
    pos[order] = np.arange(n) - starts[assign[order]]

    ctot = cpc * N_CORES
    # At blocks: At[c][s, d] = #edges(s->d) + [s==d]
    At = np.zeros((ctot, cap, cap), np.uint16)
    np.add.at(At, (assign[es], pos[es], pos[ed]), 1)
    At[assign, pos, pos] += 1
    fp8_path = int(At.max()) <= 16    # integers <= 16 are exact in e4m3

    import ml_dtypes
    x_np = {"bf16": ml_dtypes.bfloat16, "fp16": np.float16,
            "f32": np.float32, "f32r": np.float32}[_X_DT]

    Xp = np.zeros((ctot, cap, in_c), np.float32)
    Xp[assign, pos] = X
    XT_all = np.ascontiguousarray(Xp.reshape(ctot * cap, in_c).T)

    DISp = np.zeros((ctot, cap), np.float32)
    DISp[assign, pos] = dis

    if fp8_path:
        import concourse.mybir as mybir
        At_send = At.astype(mybir.dt.np(mybir.dt.float8e4))
    else:
        # rare fallback: pre-scaled B^T blocks in the compute dtype
        At_send = (At.astype(np.float32)
                   * DISp[:, :, None] * DISp[:, None, :]).astype(x_np)
    # [c, s, d] -> [c, p, so, d] so each partition row is one 2KB run
    sch = cap // P
    At_send = np.ascontiguousarray(
        At_send.reshape(-1, sch, P, cap).transpose(0, 2, 1, 3))
    DIS_send = np.ascontiguousarray(
        DISp.reshape(-1, sch, P).transpose(2, 0, 1))  # [128, ctot, sch]

    nodes = cpc * cap
    in_maps = []
    for i in range(N_CORES):
        in_maps.append({
            "XT": np.ascontiguousarray(
                XT_all[:, i * nodes:(i + 1) * nodes]).astype(x_np),
            "Wt": W.astype(np.float32).astype(x_np),
            "AT": At_send[i * cpc:(i + 1) * cpc],
            "DIS": np.ascontiguousarray(
                DIS_send[:, i * cpc:(i + 1) * cpc]),
        })
    return in_maps, fp8_path, (cpc, cap, has_edge, pos, dis)


def _run(inputs, trace=False, tmpdir=None):
    from concourse.bass_utils import run_bass_kernel_spmd

    X = np.asarray(inputs["X"], np.float32)
    W = np.asarray(inputs["W"], np.float32)
    b = np.asarray(inputs["b"], np.float32)
    assign = np.asarray(inputs["assign"])
    full_ei = np.asarray(inputs["full_ei"])

    n, in_c = X.shape
    f_out = W.shape[1]
    in_maps, fp8_path, (cpc, cap, has_edge, pos, dis) = _host_prep(
        X, W, b, assign, full_ei)
    nc = _build_program(cpc, cap, in_c, f_out, fp8_path)

    res = run_bass_kernel_spmd(
        nc, in_maps, core_ids=list(range(N_CORES)),
        trace=trace, tmpdir=tmpdir,
    )
    # YT: [core][cpc, f_out, cap]; row n lives at [core, lc, :, pos]
    YTdev = np.stack([res.results[i]["YT"] for i in range(N_CORES)])
    if YTdev.dtype != np.float32:
        YTdev = YTdev.astype(np.float32)

    c = assign.astype(np.int64)
    core = c // cpc
    lc = c % cpc
    Y = YTdev[core, lc, :, pos]
    if fp8_path:
        Y *= dis[:, None]
    Y += b[None, :].astype(np.float32)
    miss = ~has_edge[c]
    if miss.any():
        Y[miss] = X[miss]
    return Y, res


def kernel(**inputs) -> np.ndarray:
    Y, _ = _run(inputs)
    return Y
